# revision 12
# baseline (speedup 1.0000x reference)
"""Trainium2 kernel for nn_Dense_Q_MulIn1Out_Conv1D.

The reference "quantum conv" circuit is linear in the state vector, so the
whole circuit collapses to a fixed symmetric 128x128 quadratic form A:

    out[n] = (v_n^T A v_n) / (||v_n||^2 + 1e-12)

with v_n the im2col patch (16 ch x 8 taps, permuted k-major).  Host-side we
eigendecompose A = Q diag(lam) Q^T (Q orthogonal).  With y = Q^T v:

    num = sum_k lam_k y_k^2      den = ||v||^2 = sum_k y_k^2

so ONE matmul pass (Y = Q^T V), one elementwise square S = Y*Y, and ONE
reduction matmul with a [lam | ones] selector stationary produce both
numerator and denominator.  Everything on-chip runs in fp16 (tolerance is
2e-2): x is shipped as fp16, halving the im2col DMA traffic.

DMA notes (measured): each HWDGE queue processes descriptors serially at
~25ns + bytes/150GBps, so the im2col uses [128, 2048] pieces (4KB
descriptors, the packet-size cap) split across the two HWDGE rings, and
consts are folded to [64, 448] on the wire (64 big descriptors instead of
128 tiny ones) and unfolded with one DVE copy.  The PE doubles its clock
after ~3us of continuous busy, so dummy warm-up matmuls run while the
first im2col pieces stream in.
"""

import numpy as np

_DIM = 512
_D = 128
_K = 8
_C = 16
_NQ = 9
_B = 16
_L = 4096
_L_OUT = _L - _K + 1  # 4089
_N_CORES = 8
_B_PER_CORE = _B // _N_CORES  # 2
_NCHUNK = 8  # 512-column chunks per batch
_CHUNK = 512

# k-major patch permutation: new index p = k*16 + c  <->  old index c*8 + k
_PERM = np.array([(p % _C) * _K + (p // _C) for p in range(_D)])


def _apply_ry_layer(psi, angles):
    # psi [N, DIM] float64; matches reference._apply_ry_layer
    for q in range(_NQ):
        half = angles[q] * 0.5
        c, s = np.cos(half), np.sin(half)
        left = 2 ** q
        p = psi.reshape(-1, left, 2, _DIM // (2 ** (q + 1)))
        a, b = p[:, :, 0, :].copy(), p[:, :, 1, :].copy()
        psi = np.stack([c * a - s * b, s * a + c * b], axis=2).reshape(-1, _DIM)
    return psi


def _build_qlam(entangle_matrix, theta):
    """Eigendecomposition of the k-major-permuted 128x128 quadratic form."""
    U = np.asarray(entangle_matrix, dtype=np.float64)
    th = np.asarray(theta, dtype=np.float64)
    psi = np.eye(_DIM, dtype=np.float64)
    for l in range(th.shape[0]):
        psi = _apply_ry_layer(psi, th[l])
        psi = psi @ U.T
    M = psi.T  # state map: s -> M s
    z = np.concatenate([np.ones(_DIM // 2), -np.ones(_DIM // 2)])
    Md = M[:, :_D]
    A = Md.T @ (z[:, None] * Md)
    A_km = A[np.ix_(_PERM, _PERM)]
    lam, Q = np.linalg.eigh(A_km)
    return Q, lam


_NC_CACHE = {}


def _build_nc():
    import concourse.tile as tile
    from concourse import bacc, mybir

    F32 = mybir.dt.float32
    F16 = mybir.dt.float16
    AF = mybir.ActivationFunctionType

    nc = bacc.Bacc(
        "TRN2",
        target_bir_lowering=False,
        debug=False,
        num_devices=_N_CORES,
    )
    # flat fp16 x + 8 pad elements so the im2col window never reads OOB
    x = nc.dram_tensor(
        "x", [_B_PER_CORE * _C * _L + _K], F16, kind="ExternalInput"
    ).ap()
    # consts (folded): row r, cols 0:224 = [Q | T][r, :]; cols 224:448 =
    # [Q | T][64 + r, :].  T [128, 96]: col 48 = lam, col 80 = ones; the
    # window T[:, 48-j : 96-j] puts lam at within-window col j and ones at
    # col 32+j -> reduction matmul writes num to partition j, den to
    # partition 32+j of the per-batch [48, 512] PSUM tile.
    consts = nc.dram_tensor(
        "consts", [64, 2 * (_D + 96)], F16, kind="ExternalInput"
    ).ap()
    out = nc.dram_tensor(
        "out", [_B_PER_CORE * _NCHUNK, _CHUNK], F32, kind="ExternalOutput"
    ).ap()

    with tile.TileContext(nc) as tc:
        from contextlib import ExitStack

        with ExitStack() as ctx:
            const_pool = ctx.enter_context(tc.tile_pool(name="const", bufs=1))
            v_pool = ctx.enter_context(tc.tile_pool(name="v", bufs=4))
            s_pool = ctx.enter_context(tc.tile_pool(name="s", bufs=8))
            y_pool = ctx.enter_context(tc.tile_pool(name="y", bufs=3, space="PSUM"))
            red_pool = ctx.enter_context(tc.tile_pool(name="red", bufs=2, space="PSUM"))
            o_pool = ctx.enter_context(tc.tile_pool(name="o", bufs=2))

            CW = _D + 96  # 224
            c_sb = const_pool.tile([_D, 2 * CW], F16)
            # Split the folded consts across both rings so the V pieces
            # behind them start at the same time; one DVE copy unfolds
            # rows 64:128.
            nc.sync.dma_start(c_sb[0:32, :], consts[0:32, :])
            nc.scalar.dma_start(c_sb[32:64, :], consts[32:64, :])
            nc.vector.tensor_copy(c_sb[64:128, 0:CW], c_sb[0:64, CW:2 * CW])
            q_sb = c_sb[:, :_D]
            t_sb = c_sb[:, _D:CW]

            from bass_rust import AP as RawAP

            _H = 2048  # im2col piece width: [128, 2048] -> 4KB descriptors
            # v tiles [128, 2049]: partition pitch 4098B can't be coalesced
            # with the 4096B column runs by the DMA AP balancer (a flat run
            # crossing SBUF partitions is invalid).
            # Ring plan: queue behind sync flows immediately; the scalar
            # queue shows a ~3.4us arbitration gap after its first piece,
            # so it carries batch 1 (needed later).
            vh = []  # vh[b*2 + h] -> column-half tile
            for b in range(_B_PER_CORE):
                for h in range(2):
                    v = v_pool.tile([_D, _H + 1], F16, tag="v")
                    vh.append(v)
                    # dst partition (k*16+c), col n <- x[b, c, h*2048+n+k];
                    # cols >= L_OUT pick up garbage (host discards).
                    srcap = RawAP(
                        tensor=x.tensor, offset=b * _C * _L + h * _H,
                        ap=[[1, _K], [_L, _C], [1, _H]],
                    )
                    eng = nc.sync if b == 0 else nc.scalar
                    eng.dma_start(v[:, :_H], srcap)

            red0 = red_pool.tile([48, _CHUNK], F32, tag="red")
            red1 = red_pool.tile([48, _CHUNK], F32, tag="red")
            reds = [red0, red1]

            # PE p-state warm-up: ~3.5us of dummy matmuls while the first
            # im2col pieces stream in; after ~3us of continuous busy the PE
            # doubles its clock, so the real matmuls run at full rate.
            # Scratch target: batch 1's red bank, reset later by its
            # start=True matmul.
            for _ in range(9):
                nc.tensor.matmul(
                    reds[1][:, :448], t_sb[:, :48], c_sb[:, :448],
                    start=True, stop=True,
                )

            s_tiles = [None] * 8  # per chunk-pair [128, 1024] fp16

            def emit_red(b, j):
                nc.tensor.matmul(
                    reds[b][:],
                    t_sb[:, 48 - j: 96 - j],
                    s_tiles[(b * _NCHUNK + j) // 2][:, (j % 2) * _CHUNK:
                                                    (j % 2) * _CHUNK + _CHUNK],
                    start=(j == 0), stop=(j == 7),
                    skip_group_check=True,
                )

            def emit_epilogue(b):
                den_sb = o_pool.tile([8, _CHUNK], F32, tag="den")
                nc.vector.tensor_scalar_add(den_sb[:], reds[b][32:40, :], 1e-12)
                rden = o_pool.tile([8, _CHUNK], F32, tag="rden")
                nc.vector.reciprocal_approx_fast(rden[:], den_sb[:])
                out_sb = o_pool.tile([8, _CHUNK], F32, tag="outsb")
                nc.vector.tensor_mul(out_sb[:], reds[b][0:8, :], rden[:])
                nc.sync.dma_start(out[b * _NCHUNK: (b + 1) * _NCHUNK, :], out_sb[:])

            # Pair order follows DMA arrival: b0 col-halves land first on
            # the sync queue, b1 on the (gappy) scalar queue.
            order = [(0, 0), (0, 1), (1, 0), (1, 1),
                     (0, 2), (0, 3), (1, 2), (1, 3)]
            prev = None
            for b, pair in order:
                vt = vh[b * 2 + pair // 2]
                base = (pair % 2) * 1024
                y = y_pool.tile([_D, 1024], F32)
                for s in range(2):
                    nc.tensor.matmul(
                        y[:, s * _CHUNK: (s + 1) * _CHUNK],
                        q_sb,
                        vt[:, base + s * _CHUNK: base + (s + 1) * _CHUNK],
                        start=True, stop=True,
                    )
                st = s_pool.tile([_D, 1024], F16, tag="s")
                s_tiles[b * 4 + pair] = st
                nc.scalar.activation(st[:], y[:], AF.Square)
                if prev is not None:
                    pb, pp = prev
                    emit_red(pb, pp * 2)
                    emit_red(pb, pp * 2 + 1)
                    if pp == 3:
                        emit_epilogue(pb)
                prev = (b, pair)
            emit_red(1, 6)
            emit_red(1, 7)
            emit_epilogue(1)

    nc.compile()
    return nc


def get_nc():
    if "nc" not in _NC_CACHE:
        _NC_CACHE["nc"] = _build_nc()
    return _NC_CACHE["nc"]


def kernel(x, entangle_matrix, theta, _trace=False, **trace_kwargs):
    from concourse.bass_utils import run_bass_kernel_spmd

    x16 = np.asarray(x, dtype=np.float16)
    Q, lam = _build_qlam(entangle_matrix, theta)
    t = np.zeros((_D, 96), dtype=np.float64)
    t[:, 48] = lam
    t[:, 80] = 1.0
    full = np.concatenate([Q, t], axis=1).astype(np.float16)  # [128, 224]
    consts = np.ascontiguousarray(
        np.concatenate([full[0:64, :], full[64:128, :]], axis=1)
    )  # folded [64, 448]

    nc = get_nc()
    pad = np.zeros(_K, dtype=np.float16)
    in_maps = [
        {
            "x": np.concatenate(
                [x16[i * _B_PER_CORE: (i + 1) * _B_PER_CORE].reshape(-1), pad]
            ),
            "consts": consts,
        }
        for i in range(_N_CORES)
    ]
    res = run_bass_kernel_spmd(
        nc, in_maps, list(range(_N_CORES)), trace=_trace, **trace_kwargs
    )
    outs = []
    for i in range(_N_CORES):
        o = np.asarray(res.results[i]["out"], dtype=np.float32)
        outs.append(o.reshape(_B_PER_CORE, _NCHUNK * _CHUNK)[:, :_L_OUT])
    full_out = np.concatenate(outs, axis=0).reshape(_B, 1, 1, _L_OUT)
    if _trace:
        kernel._last_results = res
    return full_out


# revision 16
# speedup vs baseline: 1.1629x; 1.1629x over previous
"""Trainium2 kernel for nn_Dense_Q_MulIn1Out_Conv1D.

The reference "quantum conv" circuit is linear in the state vector, so the
whole circuit collapses to a fixed symmetric 128x128 quadratic form A:

    out[n] = (v_n^T A v_n) / (||v_n||^2 + 1e-12)

with v_n the im2col patch (16 ch x 8 taps, permuted k-major).  Host-side we
eigendecompose A = Q diag(lam) Q^T (Q orthogonal).  With y = Q^T v:

    num = sum_k lam_k y_k^2      den = ||v||^2 = sum_k y_k^2

so ONE matmul pass (Y = Q^T V), one elementwise square S = Y*Y, and ONE
reduction matmul with a [lam | ones] selector stationary produce both
numerator and denominator.  Everything on-chip runs in fp16 (tolerance is
2e-2): x is shipped as fp16, halving the im2col DMA traffic.

DMA notes (measured): each HWDGE queue processes descriptors serially at
~25ns + bytes/150GBps, so the im2col uses [128, 2048] pieces (4KB
descriptors, the packet-size cap) split across the two HWDGE rings, and
consts are folded to [64, 448] on the wire (64 big descriptors instead of
128 tiny ones) and unfolded with one DVE copy.  The PE doubles its clock
after ~3us of continuous busy, so dummy warm-up matmuls run while the
first im2col pieces stream in.
"""

import numpy as np

_DIM = 512
_D = 128
_K = 8
_C = 16
_NQ = 9
_B = 16
_L = 4096
_L_OUT = _L - _K + 1  # 4089
_N_CORES = 8
_B_PER_CORE = _B // _N_CORES  # 2
_NCHUNK = 8  # 512-column chunks per batch
_CHUNK = 512

# k-major patch permutation: new index p = k*16 + c  <->  old index c*8 + k
_PERM = np.array([(p % _C) * _K + (p // _C) for p in range(_D)])


def _apply_ry_layer(psi, angles):
    # psi [N, DIM] float64; matches reference._apply_ry_layer
    for q in range(_NQ):
        half = angles[q] * 0.5
        c, s = np.cos(half), np.sin(half)
        left = 2 ** q
        p = psi.reshape(-1, left, 2, _DIM // (2 ** (q + 1)))
        a, b = p[:, :, 0, :].copy(), p[:, :, 1, :].copy()
        psi = np.stack([c * a - s * b, s * a + c * b], axis=2).reshape(-1, _DIM)
    return psi


def _build_qlam(entangle_matrix, theta):
    """Eigendecomposition of the k-major-permuted 128x128 quadratic form."""
    U = np.asarray(entangle_matrix, dtype=np.float64)
    th = np.asarray(theta, dtype=np.float64)
    psi = np.eye(_DIM, dtype=np.float64)
    for l in range(th.shape[0]):
        psi = _apply_ry_layer(psi, th[l])
        psi = psi @ U.T
    M = psi.T  # state map: s -> M s
    z = np.concatenate([np.ones(_DIM // 2), -np.ones(_DIM // 2)])
    Md = M[:, :_D]
    A = Md.T @ (z[:, None] * Md)
    A_km = A[np.ix_(_PERM, _PERM)]
    lam, Q = np.linalg.eigh(A_km)
    return Q, lam


_NC_CACHE = {}


def _build_nc():
    import concourse.tile as tile
    from concourse import bacc, mybir

    F32 = mybir.dt.float32
    F16 = mybir.dt.float16
    AF = mybir.ActivationFunctionType

    nc = bacc.Bacc(
        "TRN2",
        target_bir_lowering=False,
        debug=False,
        num_devices=_N_CORES,
    )
    # flat fp16 x + 8 pad elements so the im2col window never reads OOB
    x = nc.dram_tensor(
        "x", [_B_PER_CORE * _C * _L + _K], F16, kind="ExternalInput"
    ).ap()
    # consts (folded): row r, cols 0:224 = [Q | T][r, :]; cols 224:448 =
    # [Q | T][64 + r, :].  T [128, 96]: col 48 = lam, col 80 = ones; the
    # window T[:, 48-j : 96-j] puts lam at within-window col j and ones at
    # col 32+j -> reduction matmul writes num to partition j, den to
    # partition 32+j of the per-batch [48, 512] PSUM tile.
    consts = nc.dram_tensor(
        "consts", [64, 2 * (_D + 96)], F16, kind="ExternalInput"
    ).ap()
    out = nc.dram_tensor(
        "out", [_B_PER_CORE * _NCHUNK, _CHUNK], F32, kind="ExternalOutput"
    ).ap()

    with tile.TileContext(nc) as tc:
        from contextlib import ExitStack

        with ExitStack() as ctx:
            const_pool = ctx.enter_context(tc.tile_pool(name="const", bufs=1))
            v_pool = ctx.enter_context(tc.tile_pool(name="v", bufs=4))
            s_pool = ctx.enter_context(tc.tile_pool(name="s", bufs=8))
            y_pool = ctx.enter_context(tc.tile_pool(name="y", bufs=3, space="PSUM"))
            red_pool = ctx.enter_context(tc.tile_pool(name="red", bufs=2, space="PSUM"))
            o_pool = ctx.enter_context(tc.tile_pool(name="o", bufs=2))

            CW = _D + 96  # 224
            c_sb = const_pool.tile([_D, 2 * CW], F16)
            # Split the folded consts across both rings so the V pieces
            # behind them start at the same time; one DVE copy unfolds
            # rows 64:128.
            nc.sync.dma_start(c_sb[0:32, :], consts[0:32, :])
            nc.scalar.dma_start(c_sb[32:64, :], consts[32:64, :])
            nc.vector.tensor_copy(c_sb[64:128, 0:CW], c_sb[0:64, CW:2 * CW])
            q_sb = c_sb[:, :_D]
            t_sb = c_sb[:, _D:CW]

            from bass_rust import AP as RawAP

            _H = 2048  # im2col piece width: [128, 2048] -> 4KB descriptors
            # v tiles [128, 2049]: partition pitch 4098B can't be coalesced
            # with the 4096B column runs by the DMA AP balancer (a flat run
            # crossing SBUF partitions is invalid).
            # Ring plan: queue behind sync flows immediately; the scalar
            # queue shows a ~3.4us arbitration gap after its first piece,
            # so it carries batch 1 (needed later).
            vh = []  # vh[b*2 + h] -> column-half tile
            ring = {(0, 0): nc.sync, (0, 1): nc.scalar,
                    (1, 0): nc.sync, (1, 1): nc.gpsimd}
            for b in range(_B_PER_CORE):
                for h in range(2):
                    v = v_pool.tile([_D, _H + 1], F16, tag="v")
                    vh.append(v)
                    # dst partition (k*16+c), col n <- x[b, c, h*2048+n+k];
                    # cols >= L_OUT pick up garbage (host discards).
                    srcap = RawAP(
                        tensor=x.tensor, offset=b * _C * _L + h * _H,
                        ap=[[1, _K], [_L, _C], [1, _H]],
                    )
                    ring[(b, h)].dma_start(v[:, :_H], srcap)

            red0 = red_pool.tile([48, _CHUNK], F32, tag="red")
            red1 = red_pool.tile([48, _CHUNK], F32, tag="red")
            reds = [red0, red1]

            # PE p-state warm-up: ~3.5us of dummy matmuls while the first
            # im2col pieces stream in; after ~3us of continuous busy the PE
            # doubles its clock, so the real matmuls run at full rate.
            # Scratch target: batch 1's red bank, reset later by its
            # start=True matmul.
            for _ in range(9):
                nc.tensor.matmul(
                    reds[1][:, :448], t_sb[:, :48], c_sb[:, :448],
                    start=True, stop=True,
                )

            s_tiles = [None] * 8  # per chunk-pair [128, 1024] fp16
            red_count = [0, 0]  # reds emitted per batch (start/stop keying)

            def emit_red(b, j):
                nc.tensor.matmul(
                    reds[b][:],
                    t_sb[:, 48 - j: 96 - j],
                    s_tiles[(b * _NCHUNK + j) // 2][:, (j % 2) * _CHUNK:
                                                    (j % 2) * _CHUNK + _CHUNK],
                    start=(red_count[b] == 0), stop=(red_count[b] == 7),
                    skip_group_check=True,
                )
                red_count[b] += 1

            def emit_epilogue(b):
                den_sb = o_pool.tile([8, _CHUNK], F32, tag="den")
                nc.vector.tensor_scalar_add(den_sb[:], reds[b][32:40, :], 1e-12)
                rden = o_pool.tile([8, _CHUNK], F32, tag="rden")
                nc.vector.reciprocal_approx_fast(rden[:], den_sb[:])
                out_sb = o_pool.tile([8, _CHUNK], F32, tag="outsb")
                nc.vector.tensor_mul(out_sb[:], reds[b][0:8, :], rden[:])
                nc.sync.dma_start(out[b * _NCHUNK: (b + 1) * _NCHUNK, :], out_sb[:])

            # Pair order follows DMA arrival: b0 col-halves land first on
            # the sync queue, b1 on the (gappy) scalar queue.
            order = [(0, 0), (0, 1), (0, 2), (0, 3),
                     (1, 2), (1, 3), (1, 0), (1, 1)]
            prev = None
            for b, pair in order:
                vt = vh[b * 2 + pair // 2]
                base = (pair % 2) * 1024
                y = y_pool.tile([_D, 1024], F32)
                for s in range(2):
                    nc.tensor.matmul(
                        y[:, s * _CHUNK: (s + 1) * _CHUNK],
                        q_sb,
                        vt[:, base + s * _CHUNK: base + (s + 1) * _CHUNK],
                        start=True, stop=True,
                    )
                st = s_pool.tile([_D, 1024], F16, tag="s")
                s_tiles[b * 4 + pair] = st
                nc.scalar.activation(st[:], y[:], AF.Square)
                if prev is not None:
                    pb, pp = prev
                    emit_red(pb, pp * 2)
                    emit_red(pb, pp * 2 + 1)
                    if (pb, pp) == (0, 3):
                        emit_epilogue(0)
                prev = (b, pair)
            emit_red(1, 0)
            emit_red(1, 1)
            emit_epilogue(1)

    nc.compile()
    return nc


def get_nc():
    if "nc" not in _NC_CACHE:
        _NC_CACHE["nc"] = _build_nc()
    return _NC_CACHE["nc"]


def kernel(x, entangle_matrix, theta, _trace=False, **trace_kwargs):
    from concourse.bass_utils import run_bass_kernel_spmd

    x16 = np.asarray(x, dtype=np.float16)
    Q, lam = _build_qlam(entangle_matrix, theta)
    t = np.zeros((_D, 96), dtype=np.float64)
    t[:, 48] = lam
    t[:, 80] = 1.0
    full = np.concatenate([Q, t], axis=1).astype(np.float16)  # [128, 224]
    consts = np.ascontiguousarray(
        np.concatenate([full[0:64, :], full[64:128, :]], axis=1)
    )  # folded [64, 448]

    nc = get_nc()
    pad = np.zeros(_K, dtype=np.float16)
    in_maps = [
        {
            "x": np.concatenate(
                [x16[i * _B_PER_CORE: (i + 1) * _B_PER_CORE].reshape(-1), pad]
            ),
            "consts": consts,
        }
        for i in range(_N_CORES)
    ]
    res = run_bass_kernel_spmd(
        nc, in_maps, list(range(_N_CORES)), trace=_trace, **trace_kwargs
    )
    outs = []
    for i in range(_N_CORES):
        o = np.asarray(res.results[i]["out"], dtype=np.float32)
        outs.append(o.reshape(_B_PER_CORE, _NCHUNK * _CHUNK)[:, :_L_OUT])
    full_out = np.concatenate(outs, axis=0).reshape(_B, 1, 1, _L_OUT)
    if _trace:
        kernel._last_results = res
    return full_out
